# revision 1
# baseline (speedup 1.0000x reference)
"""Distributed causal multi-head attention for one TRN2 chip (8 NeuronCores).

Sharding: batch (2) x head-groups (4 heads/core) -> 8 cores.
Core c handles batch c//4, heads [ (c%4)*4 , (c%4)*4+4 ).
Per core: QKV projections for its 4 heads, flash-style causal attention
with scores kept transposed (S^T = K @ Q^T) so the PV product needs no
transposes; V is augmented with a ones column so the softmax denominators
fall out of the same matmul (row 64 of each head's O^T psum).  Then an
AllGather of the attention output (pre-Wo, 4-core group = one batch) and
a column-sliced output projection.  Host assembles the 8 column/batch
shards.  Compute dtype bf16 (PSUM accumulation fp32), softmax in fp32.

Scheduling: the attention inner loop is software-pipelined one slot deep
(the S^T matmuls of slot t+1 are emitted BEFORE the PV matmuls of slot t)
so the in-order PE queue streams S(t+1) while PV(t) waits on the exp of
slot t.  K/V/Q projections for later chunks interleave as queued work
items with per-pass (fractional-chunk) deadlines; the AllGather-gated
output projections pop only two chunks after their AG fired (a premature
pop stalls the in-order PE queue) with the last chunk's leftovers
filling the final-AG wait in the drain.  Weights arrive as single wide
DMAs (host-packed 4KB rows; wq/wk pair-major so the first matmul waits
only on a 256KB half) and the x k-tiles load in chunk-column waves over
three DMA queues so arrival keeps ahead of consumption.  Softmax
normalization multiplies the O^T psum directly (no staging copy) with
the gpsimd-broadcast reciprocal of the ones-column sums.
"""

import sys
from collections import deque

import numpy as np

sys.path.insert(0, "/opt/trn_rl_repo")

import concourse.bass as bass  # noqa: E402
import concourse.bacc as bacc  # noqa: E402
import concourse.tile as tile  # noqa: E402
import concourse.mybir as mybir  # noqa: E402

F32 = mybir.dt.float32
BF16 = mybir.dt.bfloat16
ActFn = mybir.ActivationFunctionType

P = 128          # partition dim
CHUNK = 512      # i-chunk (matmul moving free dim, one psum bank of fp32)
DH = 64          # head dim
HPC = 4          # heads per core
HS = HPC * DH    # 256 per-core inner slice
DHA = DH + 1     # augmented head dim (ones column for softmax sums)
INNER = 1024     # total inner dim (16 heads x 64)
N_CORES = 8
GROUPS = [[0, 1, 2, 3], [4, 5, 6, 7]]


def build_nc(seq=2048, dim=1024, n_cores=N_CORES, groups=GROUPS, compile=True):
    """Build the SPMD Bass graph (identical on all cores)."""
    nch = seq // CHUNK          # i-chunks
    jpc = CHUNK // P            # j-tiles per chunk (4)
    njt = seq // P              # j-tiles
    nk = dim // P               # feature k-tiles
    nko = INNER // P            # inner k-tiles for the output projection
    grp = len(groups[0])        # replica group size (4)

    nc = bacc.Bacc("TRN2", target_bir_lowering=False, debug=False,
                   enable_asserts=False, num_devices=n_cores)

    xT = nc.dram_tensor("xT", [dim, seq], BF16, kind="ExternalInput").ap()
    # weights host-packed so each SBUF partition row is one contiguous DMA
    # row: [p, k*HS+j] = W[k*128+p, j]
    wq = nc.dram_tensor("wq", [P, nk * HS], BF16, kind="ExternalInput").ap()
    wk = nc.dram_tensor("wk", [P, nk * HS], BF16, kind="ExternalInput").ap()
    wv = nc.dram_tensor("wv", [P, nk * HS], BF16, kind="ExternalInput").ap()
    wo = nc.dram_tensor("wo", [P, nko * HS], BF16, kind="ExternalInput").ap()
    mask_c = nc.dram_tensor("mask_c", [P, 2 * P], BF16,
                            kind="ExternalInput").ap()
    outT = nc.dram_tensor("outT", [HS, seq], BF16, kind="ExternalOutput").ap()

    with tile.TileContext(nc) as tc:
        with tc.tile_pool(name="sb", bufs=1) as sb, \
             tc.tile_pool(name="ps", bufs=1, space="PSUM") as ps, \
             tc.tile_pool(name="dram", bufs=1, space="DRAM") as dram:

            # ---- load inputs ----
            xt = [sb.tile([P, seq], BF16, tag=f"xt{k}", name=f"xt{k}")
                  for k in range(nk)]
            wq_sb = sb.tile([P, nk * HS], BF16, tag="wq", name="wq")
            wk_sb = sb.tile([P, nk * HS], BF16, tag="wk", name="wk")
            wv_sb = sb.tile([P, nk * HS], BF16, tag="wv", name="wv")
            wo_sb = sb.tile([P, nko * HS], BF16, tag="wo", name="wo")
            mask_sb = sb.tile([P, 2 * P], BF16, tag="mask", name="mask")

            # weights: one wide transfer each (4KB rows); x k-tiles spread
            # over the three DMA queues in consumption order (per-transfer
            # fixed cost ~1-2us FIFO per queue is what gates the start)
            # load order: the first Q matmul needs wq + xt0, so those lead
            # their queues; the collectives warmup doorbell goes early on
            # gpsimd (before the bulky x tiles) so the CC entry barrier +
            # warmup clear the stream well before the first real AllGather
            warm_in = dram.tile([P, 4], BF16, tag="warm_i", name="warm_i")
            warm_out = dram.tile([grp * P, 4], BF16,
                                 tag="warm_o", name="warm_o")

            # x tiles load in column waves (chunk-0 columns for every k
            # first) on sync/gpsimd so consumption never outruns arrival;
            # the scalar queue carries ONLY the four pair-major wq/wk
            # halves back-to-back (chunk-0 pass B needs the pair-1 halves
            # by ~24us, so they can't sit behind bulk x transfers)
            qs = [nc.sync, nc.gpsimd]

            def ld_x(k, lo, hi):
                if lo < hi:
                    qs[k % 2].dma_start(xt[k][:, lo:hi],
                                        xT[k * P:(k + 1) * P, lo:hi])
            hw = nk * P
            nc.scalar.dma_start(wq_sb[:, 0:hw], wq[:, 0:hw])
            nc.gpsimd.dma_start(wv_sb[:], wv[:])
            ld_x(0, 0, CHUNK)
            nc.sync.dma_start(warm_in[:], xT[0:P, 0:4])
            nc.scalar.dma_start(wk_sb[:, 0:hw], wk[:, 0:hw])
            for k in range(1, nk):
                ld_x(k, 0, min(CHUNK, seq))
            nc.gpsimd.dma_start(mask_sb[:], mask_c[:])
            nc.gpsimd.collective_compute(
                "AllGather", mybir.AluOpType.bypass, replica_groups=groups,
                ins=[warm_in.opt()], outs=[warm_out.opt()])
            nc.scalar.dma_start(wq_sb[:, hw:2 * hw], wq[:, hw:2 * hw])
            nc.scalar.dma_start(wk_sb[:, hw:2 * hw], wk[:, hw:2 * hw])
            for k in range(nk):
                ld_x(k, CHUNK, min(2 * CHUNK, seq))
            for k in range(nk):
                ld_x(k, 2 * CHUNK, seq)
            nc.gpsimd.dma_start(wo_sb[:], wo[:])

            def wsl(w, k, a, b):
                return w[:, k * HS + a:k * HS + b]

            def wsl_pm(w, pair, k):
                # pair-major packed wq/wk: [p, (pair*nk + k)*128 + j]
                return w[:, (pair * nk + k) * P:(pair * nk + k + 1) * P]

            # persistent QKV results
            qt_sb = [sb.tile([P, seq], BF16, tag=f"qt{p}", name=f"qt{p}")
                     for p in range(2)]
            kt_sb = [sb.tile([P, seq], BF16, tag=f"kt{p}", name=f"kt{p}")
                     for p in range(2)]
            v_sb = [sb.tile([P, HPC * DHA], BF16, tag=f"v{j}", name=f"v{j}")
                    for j in range(njt)]
            ot_sb = [sb.tile([P, seq], BF16, tag=f"ot{p}", name=f"ot{p}")
                     for p in range(2)]

            # ---- interleavable work items (each emits one psum group) ----
            def emit_kt(pair, ch):
                pt = ps.tile([P, CHUNK], F32, tag="misc",
                             name=f"ktps{pair}_{ch}", bufs=2)
                for k in range(nk):
                    nc.tensor.matmul(
                        pt[:], lhsT=wsl_pm(wk_sb, pair, k),
                        rhs=xt[k][:, ch * CHUNK:(ch + 1) * CHUNK],
                        start=(k == 0), stop=(k == nk - 1))
                nc.vector.tensor_copy(
                    kt_sb[pair][:, ch * CHUNK:(ch + 1) * CHUNK], pt[:])

            def emit_v(jt):
                pt = ps.tile([P, HS], F32, tag="misc",
                             name=f"vps{jt}", bufs=2)
                for k in range(nk):
                    nc.tensor.matmul(
                        pt[:], lhsT=xt[k][:, jt * P:(jt + 1) * P],
                        rhs=wsl(wv_sb, k, 0, HS),
                        start=(k == 0), stop=(k == nk - 1))
                nc.vector.tensor_copy(
                    v_sb[jt].rearrange("p (h d) -> p h d", h=HPC)[:, :, 0:DH],
                    pt.rearrange("p (h d) -> p h d", h=HPC))
                nc.vector.memset(
                    v_sb[jt].rearrange("p (h d) -> p h d", h=HPC)[:, :,
                                                                  DH:DHA],
                    1.0)

            def emit_qt(pair, chh):
                pt = ps.tile([P, CHUNK], F32, tag="misc",
                             name=f"qps{pair}_{chh}", bufs=2)
                for k in range(nk):
                    nc.tensor.matmul(
                        pt[:],
                        lhsT=wsl_pm(wq_sb, pair, k),
                        rhs=xt[k][:, chh * CHUNK:(chh + 1) * CHUNK],
                        start=(k == 0), stop=(k == nk - 1))
                nc.scalar.activation(
                    qt_sb[pair][:, chh * CHUNK:(chh + 1) * CHUNK],
                    pt[:], ActFn.Copy)

            def emit_proj(ci, m, slices, korder, op_ps=None, evac=True):
                # transposed output block: outT[m*128:(m+1)*128, chunk ci]
                # = Wo[:, m-slice].T @ attT[:, chunk] over the k-tiles in
                # `korder` (a partial pass keeps op_ps alive).
                c0 = ci * CHUNK
                first = op_ps is None
                if first:
                    op_ps = ps.tile([P, CHUNK], F32, tag="misc",
                                    name=f"op{ci}_{m}", bufs=2)
                for n, k in enumerate(korder):
                    ag_t, coff = slices[k]
                    nc.tensor.matmul(
                        op_ps[:],
                        lhsT=wsl(wo_sb, k, m * P, (m + 1) * P),
                        rhs=ag_t[:, coff:coff + CHUNK],
                        start=(first and n == 0),
                        stop=(evac and n == len(korder) - 1))
                if not evac:
                    return op_ps
                o_sb = sb.tile([P, CHUNK], BF16, tag="osb",
                               name=f"o{ci}_{m}", bufs=2)
                nc.vector.tensor_copy(o_sb[:], op_ps[:])
                nc.sync.dma_start(
                    outT[m * P:(m + 1) * P, c0:c0 + CHUNK], o_sb[:])
                return None

            work_early = deque()   # (target_chunk, fn): KT/V/Q for later
            work_late = deque()    # (chunk, fn): AllGather-gated projections

            def pop_work(in_late_window, cur_ci=None):
                # a popped projection whose AllGather hasn't landed stalls
                # the in-order PE queue: pop proj(ci) only in the late
                # window of chunk ci+2, or in the final drain — where the
                # ungated items ahead of it fill the last-AG wait anyway
                if work_early:
                    work_early.popleft()[1]()
                    if len(work_early) > 4:
                        work_early.popleft()[1]()
                elif in_late_window and work_late and cur_ci is None:
                    # all AG-gated projections run in the final drain: every
                    # AG but the last is long complete by then, so they are
                    # ready PE work spanning the last AG's flight
                    work_late.popleft()[1]()

            def emit_ag_full(ci, bounce_in):
                # one AllGather for both head pairs of chunk `ci` (256KB —
                # amortizes the ncfw floor; rank-major rows land so that
                # gathered row-block k*128 is exactly attT k-tile k)
                bounce_out = dram.tile([grp * 2 * P, CHUNK], BF16,
                                       tag="boutf", name=f"boutf{ci}", bufs=2)
                nc.gpsimd.collective_compute(
                    "AllGather", mybir.AluOpType.bypass,
                    replica_groups=groups,
                    ins=[bounce_in.opt()], outs=[bounce_out.opt()])
                tiles = {}
                for k in range(nko):
                    t = sb.tile([P, CHUNK], BF16, tag=f"ag{k}",
                                name=f"ag{ci}_{k}", bufs=2)
                    nc.sync.dma_start(t[:],
                                      bounce_out[k * P:(k + 1) * P, :])
                    tiles[k] = t
                return tiles

            def emit_ag_pair(ci, pair):
                # half AllGather (one head pair) of chunk `ci` — fired right
                # after that pair's normalize so pair A overlaps the second
                # attention pass.  Gathered k-tiles land at k = 2r+pair.
                c0 = ci * CHUNK
                bounce_in = dram.tile([P, CHUNK], BF16, tag=f"binh{pair}",
                                      name=f"binh{ci}_{pair}", bufs=2)
                bounce_out = dram.tile([grp * P, CHUNK], BF16,
                                       tag=f"bouth{pair}",
                                       name=f"bouth{ci}_{pair}", bufs=2)
                nc.sync.dma_start(bounce_in[:], ot_sb[pair][:, c0:c0 + CHUNK])
                nc.gpsimd.collective_compute(
                    "AllGather", mybir.AluOpType.bypass,
                    replica_groups=groups,
                    ins=[bounce_in.opt()], outs=[bounce_out.opt()])
                tiles = {}
                for r in range(grp):
                    k = 2 * r + pair
                    t = sb.tile([P, CHUNK], BF16, tag=f"ag{k}",
                                name=f"ag{ci}_{k}", bufs=2)
                    nc.sync.dma_start(t[:], bounce_out[r * P:(r + 1) * P, :])
                    tiles[k] = t
                return tiles

            # ---- upfront: just enough for chunk-0 pass A to start (pair-0
            # Q/K + first V tiles); everything else flows through the work
            # queue.  Fractional targets gate pass-B deps (drained at the
            # pass-B boundary) vs next-chunk deps (drained at chunk start).
            emit_qt(0, 0)
            emit_v(0)
            emit_v(1)
            emit_kt(0, 0)
            for jt in range(2, jpc):
                work_early.append((0.5, lambda jt=jt: emit_v(jt)))
            work_early.append((0.5, lambda: emit_qt(1, 0)))
            work_early.append((0.5, lambda: emit_kt(1, 0)))

            # ---- attention chunks ----
            last_parts = {}
            for ci in range(nch):
                jt_end = jpc * (ci + 1)
                c0 = ci * CHUNK
                last = ci == nch - 1

                # anything this chunk's S matmuls depend on must be emitted
                # before the chunk starts (in-order PE queue)
                while work_early and work_early[0][0] <= ci:
                    work_early.popleft()[1]()

                if ci + 1 < nch:
                    # pass-A deps of chunk ci+1 (pair-0 Q/K, new V tiles)
                    # then pass-B-only deps at the half-chunk target
                    work_early.append(
                        (ci + 1, lambda ch=ci + 1: emit_qt(0, ch)))
                    work_early.append(
                        (ci + 1, lambda ch=ci + 1: emit_kt(0, ch)))
                    for jt in range(jpc * (ci + 1), jpc * (ci + 2)):
                        work_early.append(
                            (ci + 1, lambda jt=jt: emit_v(jt)))
                    work_early.append(
                        (ci + 1.5, lambda ch=ci + 1: emit_qt(1, ch)))
                    work_early.append(
                        (ci + 1.5, lambda ch=ci + 1: emit_kt(1, ch)))

                binf = None if last else dram.tile(
                    [2 * P, CHUNK], BF16, tag="binf", name=f"binf{ci}",
                    bufs=2)

                ot_ps = {}
                pend = [None]

                def do_pass_end(hp, ci=ci, c0=c0, last=last, binf=binf,
                                ot_ps=ot_ps):
                    # softmax normalize: rcp of each head's sum row (staged
                    # to SBUF), gpsimd broadcast, then one mul per head
                    # reading the O^T psum directly (no staging copy)
                    for h2 in range(2):
                        srow = sb.tile([1, CHUNK], F32, tag=f"sr{h2}",
                                       name=f"sr{ci}_{hp}_{h2}", bufs=2)
                        nc.vector.tensor_copy(srow[:],
                                              ot_ps[hp][h2][DH:DHA, :])
                        rcp = sb.tile([1, CHUNK], F32, tag=f"rcp{h2}",
                                      name=f"rcp{ci}_{hp}_{h2}", bufs=2)
                        nc.vector.reciprocal_approx_fast(rcp[:], srow[:])
                        bc_sb = sb.tile([DH, CHUNK], F32, tag=f"bc{h2}",
                                        name=f"bc{ci}_{hp}_{h2}", bufs=2)
                        nc.gpsimd.partition_broadcast(bc_sb[:], rcp[:],
                                                      channels=DH)
                        nc.vector.tensor_mul(
                            ot_sb[hp][h2 * DH:(h2 + 1) * DH, c0:c0 + CHUNK],
                            ot_ps[hp][h2][0:DH, :],
                            bc_sb[:])
                    if last:
                        last_parts.update(emit_ag_pair(ci, hp))
                    else:
                        nc.sync.dma_start(
                            binf[hp * P:(hp + 1) * P, :],
                            ot_sb[hp][:, c0:c0 + CHUNK])

                def flush(jt_end=jt_end, last=last, ot_ps=ot_ps, pend=pend):
                    if pend[0] is None:
                        return
                    hp, jt, es, rel = pend[0]
                    pend[0] = None
                    for h2 in range(2):
                        h = 2 * hp + h2
                        nc.tensor.matmul(
                            ot_ps[hp][h2][:, rel:CHUNK],
                            lhsT=v_sb[jt][:, h * DHA:(h + 1) * DHA],
                            rhs=es[:, h2 * CHUNK + rel:(h2 + 1) * CHUNK],
                            start=(jt == 0), stop=(jt == jt_end - 1))
                    if jt == jt_end - 1:
                        do_pass_end(hp)
                    # the last chunk keeps its leftover projections for the
                    # final drain, where they fill the last-AG flight time
                    pop_work(
                        in_late_window=(hp == 1 and jt >= jt_end - 2
                                        and not last),
                        cur_ci=ci)

                for hp in range(2):
                    for jt in range(jt_end):
                        if jt == 0:
                            if hp == 1:
                                # pass-B S matmuls need pair-1 Q/K emitted
                                # before them in the in-order PE queue
                                while (work_early
                                       and work_early[0][0] <= ci + 0.5):
                                    work_early.popleft()[1]()
                            ot_ps[hp] = [
                                ps.tile([DHA, CHUNK], F32, tag=f"ot{h2}",
                                        name=f"ot{ci}_{hp}_{h2}", bufs=1)
                                for h2 in range(2)]
                        rel = max(0, (jt - jpc * ci)) * P
                        diag = jt >= jpc * ci

                        s2 = ps.tile([P, 2 * CHUNK], F32, tag="s2",
                                     name=f"s{ci}_{hp}_{jt}", bufs=2)
                        es = sb.tile([P, 2 * CHUNK], BF16, tag="es",
                                     name=f"es{ci}_{hp}_{jt}", bufs=4)

                        for h2 in range(2):
                            # S^T tile = K_h @ Q_h^T (row-tiled, K=64; the
                            # two heads run concurrently in the PE array)
                            nc.tensor.matmul(
                                s2[:, h2 * CHUNK + rel:(h2 + 1) * CHUNK],
                                lhsT=kt_sb[hp][h2 * DH:(h2 + 1) * DH,
                                               jt * P:(jt + 1) * P],
                                rhs=qt_sb[hp][h2 * DH:(h2 + 1) * DH,
                                              c0 + rel:c0 + CHUNK],
                                start=True, stop=True,
                                tile_position=(h2 * DH, 0))
                        # one exp for both heads (both psum banks); flat AP
                        # off the diagonal (3D APs cost ~190ns extra on ACT)
                        if rel == 0:
                            nc.scalar.activation(es[:], s2[:], ActFn.Exp)
                        else:
                            nc.scalar.activation(
                                es.rearrange("p (t c) -> p t c",
                                             t=2)[:, :, rel:],
                                s2.rearrange("p (t c) -> p t c",
                                             t=2)[:, :, rel:],
                                ActFn.Exp)
                        if diag:
                            nc.vector.tensor_mul(
                                es.rearrange("p (t c) -> p t c",
                                             t=2)[:, :, rel:rel + P],
                                es.rearrange("p (t c) -> p t c",
                                             t=2)[:, :, rel:rel + P],
                                mask_sb.rearrange("p (t c) -> p t c", t=2))
                        # software pipeline: PV of the previous slot issues
                        # AFTER this slot's S, so the PE never stalls on exp
                        flush()
                        pend[0] = (hp, jt, es, rel)
                flush()

                if last:
                    agt = dict(last_parts)
                    last_parts = {}
                else:
                    agt = emit_ag_full(ci, binf)
                slices = [(agt[k], 0) for k in range(nko)]
                evens = [k for k in range(nko) if k % 2 == 0]
                odds = [k for k in range(nko) if k % 2 == 1]
                nm = HS // P
                if not last:
                    for m in range(nm):
                        work_late.append(
                            (ci,
                             lambda ci=ci, m=m, s=slices, ko=list(range(nko)):
                             emit_proj(ci, m, s, ko)))
                else:
                    # split each output block's projection: the even k-tiles
                    # (from the pair-A AllGather) run while pair-B flies
                    op_tiles = {}

                    def proj_ev(ci, m, s):
                        op_tiles[m] = emit_proj(ci, m, s, evens, evac=False)

                    def proj_od(ci, m, s):
                        emit_proj(ci, m, s, odds, op_ps=op_tiles.pop(m))

                    for m in range(nm):
                        work_late.append(
                            (ci,
                             lambda ci=ci, m=m, s=slices: proj_ev(ci, m, s)))
                    for m in range(nm):
                        work_late.append(
                            (ci,
                             lambda ci=ci, m=m, s=slices: proj_od(ci, m, s)))

            while work_early or work_late:
                pop_work(in_late_window=True)

    if compile:
        nc.compile()
    return nc


def make_in_maps(x, Wq, Wk, Wv, Wo, n_cores=N_CORES):
    import ml_dtypes
    bf16 = ml_dtypes.bfloat16
    scale = np.float32(DH ** -0.5)
    # band mask for the diagonal j-tile of S^T [j,i]: keep j <= i
    # (duplicated side by side so one DVE mul covers both heads)
    mask_b = np.triu(np.ones((P, P), np.float32))
    mask2 = np.concatenate([mask_b, mask_b], axis=1).astype(bf16)

    def pack(sl):
        # [ntk*128, HS] -> [128, ntk*HS]: row p holds k-tile blocks side by
        # side so the whole weight is one contiguous-row DMA
        ntk = sl.shape[0] // P
        return np.ascontiguousarray(
            sl.reshape(ntk, P, HS).transpose(1, 0, 2).reshape(P, ntk * HS)
        ).astype(bf16)

    def pack_pm(sl):
        # pair-major: [nk*128, 2*128] -> [128, 2*nk*128] with
        # out[p, (pair*nk + k)*128 + j] = sl[k*128 + p, pair*128 + j],
        # so each head-pair's weights are one contiguous half
        ntk = sl.shape[0] // P
        return np.ascontiguousarray(
            sl.reshape(ntk, P, 2, P).transpose(1, 2, 0, 3).reshape(
                P, 2 * ntk * P)
        ).astype(bf16)

    in_maps = []
    for c in range(n_cores):
        b, r = divmod(c, 4)
        hs = r * HS
        in_maps.append({
            "xT": np.ascontiguousarray(x[b].T).astype(bf16),
            "wq": pack_pm(Wq[:, hs:hs + HS] * scale),
            "wk": pack_pm(Wk[:, hs:hs + HS]),
            "wv": pack(Wv[:, hs:hs + HS]),
            "wo": pack(Wo[:, hs:hs + HS]),
            "mask_c": mask2,
        })
    return in_maps


def assemble_out(results, B, seq, n_cores=N_CORES):
    out = np.empty((B, seq, INNER), np.float32)
    for c in range(n_cores):
        b, r = divmod(c, 4)
        out[b][:, r * HS:(r + 1) * HS] = results[c]["outT"].T.astype(
            np.float32)
    return out


_NC_CACHE = {}


def kernel(x, Wq, Wk, Wv, Wo):
    from concourse import bass_utils
    x = np.asarray(x, np.float32)
    B, seq, dim = x.shape
    key = (seq, dim)
    if key not in _NC_CACHE:
        _NC_CACHE[key] = build_nc(seq=seq, dim=dim)
    nc = _NC_CACHE[key]
    in_maps = make_in_maps(x, np.asarray(Wq, np.float32),
                           np.asarray(Wk, np.float32),
                           np.asarray(Wv, np.float32),
                           np.asarray(Wo, np.float32))
    res = bass_utils.run_bass_kernel_spmd(
        nc, in_maps, core_ids=list(range(N_CORES)))
    return assemble_out(res.results, B, seq)

